# revision 1
# baseline (speedup 1.0000x reference)
"""Trainium2 Bass kernel for CoarseBlockAttention.

Reference computation (per batch b, with x: (C, H, W), C=512, H=W=64, S=4):
  x_avg  = 4x4 block means of x            -> (nb=256, C)  [unfold order bh*16+bw]
  Q = x_avg @ Wq.T + bq ; K = x_avg @ Wk.T + bk
  A = softmax(Q K^T / sqrt(C))             -> (256, 256)
  V = x_flat @ Wv.T + bv  (x_flat: flat row-major pixels, (4096, C))
  Vsum = V summed over groups of 16 consecutive flat pixels -> (256, C)
  out_small = A @ Vsum                     -> (256, C)
  out[c, p] = out_small[p // 16, c]        (repeat_interleave by 16)

Algebraic restructuring used here (all exact):
  * Vsum = Xsum @ Wv.T + 16*bv  with Xsum the group-of-16 pixel sums of x
    (linearity) -- shrinks the V projection by 16x.
  * Softmax rows of A sum to 1 => A @ (1 (16 bv)^T) = 1 (16 bv)^T, so the V
    bias is a per-channel constant added to out_small at the end.
  * Q K^T = xa (Wq^T Wk) xa^T + [row-const terms] + 1 (u . xa[m])^T with
    u = Wk^T bq.  Row-constant terms cancel in softmax.  So only the fused
    matrix W2 = Wq^T Wk and vector u are needed; bq/bk never materialize.
  * The 1/16 block-mean scaling and 1/sqrt(C) logit scaling are folded into
    W2 and u on the host.

Device layout (per core = one batch element, 8 cores data-parallel over B=8):
  XaT[c, n] : 4x4 block sums   (C on partitions, 4 chunks of 128)
  XsT[c, m] : 1x16 run sums    (same layout)
  G = W2s @ XaT        (PE, contracting c' chunks)       -> (c, 256)
  L = XaT^T G + 1 cs^T (PE)                              -> (n, 256) logits
  A = softmax rows (DVE reduce max / ACT exp / DVE reciprocal+scale)
  At = A^T (PE transpose)                                 -> (m, n)
  Vs = XsT^T WvT       (PE)                              -> (m, o=512)
  outT = Vs^T At  (PE) -> (o, n); ACT adds 16*bv and expands 16x along free
  dim (broadcast read from PSUM) before the contiguous DMA store.
"""

import math
from contextlib import ExitStack

import numpy as np

import concourse.bacc as bacc
import concourse.bass as bass
import concourse.mybir as mybir
import concourse.tile as tile
from concourse._compat import get_trn_type
from concourse.bass_utils import run_bass_kernel_spmd
from concourse.masks import make_identity

B, C, H, W, S = 8, 512, 64, 64, 4
HW = H * W          # 4096
NB = (H // S) * (W // S)  # 256
P = 128
KC = C // P         # 4 contraction/channel chunks
F32 = mybir.dt.float32
AX = mybir.AxisListType
AF = mybir.ActivationFunctionType


def _kernel_body(tc: "tile.TileContext", ctx, out, xb, w2t, wvt, us, b16):
    nc = tc.nc
    # fp32r: 1 cycle/row on PE (vs 4 for fp32).  walrus requires every fp32r
    # matmul operand to be *produced* with dtype float32r, so the operand
    # tiles are declared float32r and the producing engine rounds on write.
    FR = mybir.dt.float32r
    r = lambda ap: ap

    singles = ctx.enter_context(tc.tile_pool(name="singles", bufs=1))
    xpool = ctx.enter_context(tc.tile_pool(name="xpool", bufs=3))
    s1pool = ctx.enter_context(tc.tile_pool(name="s1pool", bufs=2))
    prpool = ctx.enter_context(tc.tile_pool(name="prpool", bufs=2))
    expool = ctx.enter_context(tc.tile_pool(name="expool", bufs=2))

    # Warm the ACT exp table during the DMA-in phase.
    dummy = singles.tile([P, 1], F32, name="dummy")
    nc.vector.memset(dummy, 0.0)
    nc.scalar.activation(dummy, dummy, AF.Exp)

    ident = singles.tile([P, P], F32, name="ident")
    make_identity(nc, ident)
    ones1_f = singles.tile([1, P], F32, name="ones1_f")
    nc.vector.memset(ones1_f, 1.0)
    ones1 = singles.tile([1, P], FR, name="ones1")
    nc.vector.tensor_copy(ones1, ones1_f)

    w2_sb = singles.tile([P, KC, C], FR, name="w2_sb")
    wv_sb = singles.tile([P, KC, C], FR, name="wv_sb")
    w2_d = w2t.rearrange("(k p) c -> p k c", p=P)
    wv_d = wvt.rearrange("(k p) c -> p k c", p=P)
    us_sb = singles.tile([P, KC], FR, name="us_sb")
    b16_sb = singles.tile([P, KC], F32, name="b16_sb")

    xa_sb = singles.tile([P, KC, NB], FR, name="xa_sb")  # 4x4 block sums^T
    xs_sb = singles.tile([P, KC, NB], FR, name="xs_sb")  # 1x16 run sums^T

    psA = tc.alloc_tile_pool(name="psA", bufs=1, space="PSUM")
    g_ps = [psA.tile([P, NB], F32, name=f"g_ps{j}") for j in range(KC)]
    vs_ps = [psA.tile([P, C], F32, name=f"vs_ps{m}") for m in range(2)]
    cs_ps = psA.tile([1, NB], F32, name="cs_ps")

    # Streaming phase: x arrives in 1 MB half-chunk pieces; pairwise-add trees
    # produce the 4-wide sums (DVE takes piece h=0, GPSIMD piece h=1, so the
    # two engines chase the DMA stream in parallel).  Weight slices are
    # interleaved between x pieces so they don't delay the first reductions.
    PW = HW // 2  # 2048 columns per piece
    for k in range(KC):
        s1 = s1pool.tile([P, 1024], F32, name="s1")
        for h in range(2):
            x_t = xpool.tile([P, PW], F32, name="x_t")
            nc.sync.dma_start(
                out=x_t, in_=xb[k * P:(k + 1) * P, h * PW:(h + 1) * PW]
            )
            eng = nc.vector if h == 0 else nc.gpsimd
            xv = x_t.rearrange("p (q two) -> p q two", two=2)
            pr = prpool.tile([P, 1024], F32, name="pr")
            eng.tensor_add(pr, xv[:, :, 0], xv[:, :, 1])
            pv = pr.rearrange("p (q two) -> p q two", two=2)
            eng.tensor_add(s1[:, h * 512:(h + 1) * 512], pv[:, :, 0], pv[:, :, 1])
        if k == 0:
            nc.sync.dma_start(out=us_sb, in_=us.rearrange("(k p) -> p k", p=P))
            nc.sync.dma_start(out=b16_sb, in_=b16.rearrange("(k p) -> p k", p=P))
        # weight slices for this chunk's matmuls (and spares) land here
        nc.sync.dma_start(out=w2_sb[:, k, :], in_=w2_d[:, k, :])
        nc.sync.dma_start(out=wv_sb[:, k, :], in_=wv_d[:, k, :])
        with nc.allow_low_precision(reason="fp32r matmul operands"):
            # 1x16 run sums: 4 consecutive s1 entries (same h)
            nc.vector.reduce_sum(
                xs_sb[:, k, :], s1.rearrange("p (m r) -> p m r", r=4), axis=AX.X
            )
            # 4x4 block sums: 4 s1 entries strided by 16 (dh direction)
            nc.vector.reduce_sum(
                xa_sb[:, k, :],
                s1.rearrange("p (bh dh bw) -> p bh bw dh", dh=4, bw=16),
                axis=AX.X,
            )
        first, last = (k == 0), (k == KC - 1)
        for j in range(KC):
            nc.tensor.matmul(
                g_ps[j],
                lhsT=r(w2_sb[:, k, j * P:(j + 1) * P]),
                rhs=r(xa_sb[:, k, :]),
                start=first,
                stop=last,
            )
        for m in range(2):
            nc.tensor.matmul(
                vs_ps[m],
                lhsT=r(xs_sb[:, k, m * P:(m + 1) * P]),
                rhs=r(wv_sb[:, k, :]),
                start=first,
                stop=last,
            )
        nc.tensor.matmul(
            cs_ps,
            lhsT=r(us_sb[:, k:k + 1]),
            rhs=r(xa_sb[:, k, :]),
            start=first,
            stop=last,
        )

    # PSUM -> SBUF staging, split across ACT and DVE to cut the latency on the
    # critical path into the L matmuls.
    g_sb = singles.tile([P, KC, NB], FR, name="g_sb")
    for j in range(KC):
        if j < 2:
            nc.scalar.copy(g_sb[:, j, :], g_ps[j])
        else:
            nc.vector.tensor_copy(g_sb[:, j, :], g_ps[j])
    vs_sb = singles.tile([P, 2, C], FR, name="vs_sb")
    nc.scalar.copy(vs_sb[:, 0, :], vs_ps[0])
    nc.vector.tensor_copy(vs_sb[:, 1, :], vs_ps[1])
    cs_sb = singles.tile([1, NB], FR, name="cs_sb")
    nc.scalar.copy(cs_sb, cs_ps)
    psA.release()

    psB = tc.alloc_tile_pool(name="psB", bufs=1, space="PSUM")

    # Logits + softmax (row chunks of 128).
    a_sb = singles.tile([P, 2, NB], F32, name="a_sb")
    nmax = singles.tile([P, 2], F32, name="nmax")
    rsum = singles.tile([P, 2], F32, name="rsum")
    l_ps = [psB.tile([P, NB], F32, name=f"l_ps{n}") for n in range(2)]
    for n in range(2):
        for k in range(KC):
            nc.tensor.matmul(
                l_ps[n],
                lhsT=r(xa_sb[:, k, n * P:(n + 1) * P]),
                rhs=r(g_sb[:, k, :]),
                start=(k == 0),
                stop=False,
            )
        # + 1 cs^T : broadcast the column-bias row via a K=1 matmul
        nc.tensor.matmul(
            l_ps[n], lhsT=r(ones1), rhs=r(cs_sb), start=False, stop=True
        )
        nc.vector.reduce_max(nmax[:, n:n + 1], l_ps[n], axis=AX.X, negate=True)
        nc.scalar.activation(
            a_sb[:, n, :],
            l_ps[n],
            AF.Exp,
            bias=nmax[:, n:n + 1],
            accum_out=rsum[:, n:n + 1],
        )
        nc.vector.reciprocal(rsum[:, n:n + 1], rsum[:, n:n + 1])
        nc.vector.tensor_scalar_mul(a_sb[:, n, :], a_sb[:, n, :], rsum[:, n:n + 1])

    # At[m, n] = A[n, m] via PE transpose of 128x128 blocks.
    at_sb = singles.tile([P, 2, NB], FR, name="at_sb")
    for n in range(2):
        for m in range(2):
            t_ps = psB.tile([P, P], F32, name="t_ps", bufs=2)
            nc.tensor.transpose(t_ps, a_sb[:, n, m * P:(m + 1) * P], ident)
            nc.vector.tensor_copy(at_sb[:, m, n * P:(n + 1) * P], t_ps)

    # outT[o, n] = sum_m Vs[m, o] At[m, n]; then +16*bv and 16x expansion.
    o_ps = [psB.tile([P, NB], F32, name=f"o_ps{j}") for j in range(KC)]
    for j in range(KC):
        for m in range(2):
            nc.tensor.matmul(
                o_ps[j],
                lhsT=r(vs_sb[:, m, j * P:(j + 1) * P]),
                rhs=r(at_sb[:, m, :]),
                start=(m == 0),
                stop=(m == 1),
            )
        ex = expool.tile([P, HW], F32, name="ex")
        nc.scalar.activation(
            ex.rearrange("p (q s) -> p q s", s=16),
            o_ps[j].broadcast_to((P, NB, 16)),
            AF.Identity,
            bias=b16_sb[:, j:j + 1],
        )
        nc.sync.dma_start(out=out[j * P:(j + 1) * P, :], in_=ex)
    psB.release()


def _build():
    nc = bacc.Bacc(
        get_trn_type() or "TRN2", target_bir_lowering=False, debug=False
    )
    xb = nc.dram_tensor("xb", (C, HW), F32, kind="ExternalInput").ap()
    w2t = nc.dram_tensor("w2t", (C, C), mybir.dt.float32r, kind="ExternalInput").ap()
    wvt = nc.dram_tensor("wvt", (C, C), mybir.dt.float32r, kind="ExternalInput").ap()
    us = nc.dram_tensor("us", (C,), mybir.dt.float32r, kind="ExternalInput").ap()
    b16 = nc.dram_tensor("b16", (C,), F32, kind="ExternalInput").ap()
    out = nc.dram_tensor("out", (C, HW), F32, kind="ExternalOutput").ap()

    with tile.TileContext(nc) as tc:
        with ExitStack() as ctx:
            _kernel_body(tc, ctx, out, xb, w2t, wvt, us, b16)
    nc.compile()
    return nc


_CACHE: dict = {}


def _get_nc():
    if "nc" not in _CACHE:
        _CACHE["nc"] = _build()
    return _CACHE["nc"]


def _prep_inputs(x, Wq, bq, Wk, bk, Wv, bv):
    f = lambda a: np.ascontiguousarray(np.asarray(a, dtype=np.float32))
    x, Wq, bq, Wk, bk, Wv, bv = map(f, (x, Wq, bq, Wk, bk, Wv, bv))
    s = 1.0 / math.sqrt(C)
    w2t = np.ascontiguousarray((Wk.T @ Wq) * (s / 256.0)).astype(np.float32)
    usv = ((Wk.T @ bq) * (s / 16.0)).astype(np.float32)
    wvt = np.ascontiguousarray(Wv.T).astype(np.float32)
    b16 = (16.0 * bv).astype(np.float32)
    in_maps = [
        {
            "xb": np.ascontiguousarray(x[b].reshape(C, HW)),
            "w2t": w2t,
            "wvt": wvt,
            "us": usv,
            "b16": b16,
        }
        for b in range(B)
    ]
    return in_maps


def run(inputs: dict, trace: bool = False, tmpdir: str | None = None):
    """Run on 8 NeuronCores; returns (output (B,C,H,W) f32, BassKernelResults)."""
    nc = _get_nc()
    in_maps = _prep_inputs(**inputs)
    rr = run_bass_kernel_spmd(nc, in_maps, list(range(B)), trace=trace, tmpdir=tmpdir)
    out = np.stack([r["out"] for r in rr.results]).reshape(B, C, H, W)
    return out.astype(np.float32), rr


def kernel(**inputs) -> np.ndarray:
    out, _ = run(inputs, trace=False)
    return out

